# revision 1
# baseline (speedup 1.0000x reference)
"""HardNegativeMiningLoss on 8 TRN2 NeuronCores.

Data-parallel over anchor rows: core c owns rows [1024c, 1024(c+1)).
Each core holds full E^T (bf16) in SBUF, computes its [1024, 8192] sim
block with TensorE (fp32 PSUM), evacuates each 512-col chunk to bf16
SBUF via ScalarE copy, applies the semi-hard shift
u - 1000*[u >= pos_min] (GpSimd is_ge/mult + DVE add, bf16 2x mode),
extracts per-512-segment top-8 with DVE max8, merges to top-16 via
max8/match_replace/max8, and (once, at the end, so ACT never swaps
activation tables mid-loop) does the masked logsumexp with ACT Exp/Ln.
Label-derived row metadata (pos_min / pos_sim / valid) is precomputed
on host (~0.05% of FLOPs).  Host sums the per-core [128, 8] partials.
"""

import numpy as np

import concourse.bacc as bacc
import concourse.bass as bass
import concourse.mybir as mybir
import concourse.tile as tile
from concourse.bass_utils import run_bass_kernel_spmd

B = 8192
D = 512
N_CORES = 8
ROWS_PER_CORE = B // N_CORES          # 1024
N_ROW_TILES = ROWS_PER_CORE // 128    # 8
CHUNK = 512
N_CHUNKS = B // CHUNK                 # 16
TEMP = 0.07
GESHIFT = 1000.0
CORR = GESHIFT / TEMP
FP = mybir.dt.float32
BF = mybir.dt.bfloat16


def _build_program():
    nc = bacc.Bacc(None, target_bir_lowering=False)

    et_d = nc.dram_tensor("et", [D, B], BF, kind="ExternalInput")
    eloc_d = nc.dram_tensor("eloc", [D, ROWS_PER_CORE], BF, kind="ExternalInput")
    meta_d = nc.dram_tensor("rowmeta", [ROWS_PER_CORE, 4], FP, kind="ExternalInput")
    out_d = nc.dram_tensor("out", [128, N_ROW_TILES], FP, kind="ExternalOutput")

    et_v = et_d[:].rearrange("(k p) n -> k p n", p=128)       # [4,128,B]
    eloc_v = eloc_d[:].rearrange("(k p) n -> k p n", p=128)   # [4,128,1024]
    meta_v = meta_d[:].rearrange("(t p) m -> p t m", p=128)   # [128,8,4]
    NK = D // 128

    with tile.TileContext(nc) as tc:
        with (
            tc.tile_pool(name="wts", bufs=1) as wts,
            tc.tile_pool(name="upool", bufs=3) as upool,
            tc.tile_pool(name="psum", bufs=8, space="PSUM") as psp,
            tc.tile_pool(name="pen", bufs=4) as penp,
            tc.tile_pool(name="small", bufs=2) as smp,
            tc.tile_pool(name="acc", bufs=1) as accp,
        ):
            # resident inputs
            et_t = []
            for k in range(NK):
                t = wts.tile([128, B], BF, tag=f"et{k}")
                nc.sync.dma_start(t[:], et_v[k])
                et_t.append(t)
            eloc_t = []
            for k in range(NK):
                t = wts.tile([128, ROWS_PER_CORE], BF, tag=f"el{k}")
                nc.sync.dma_start(t[:], eloc_v[k])
                eloc_t.append(t)
            metas = accp.tile([128, N_ROW_TILES, 4], FP, tag="metas")
            nc.sync.dma_start(metas[:], meta_v)

            t16a = accp.tile([128, N_ROW_TILES, 16], FP, tag="t16a")
            loss_t = accp.tile([128, N_ROW_TILES], FP)

            for rt in range(N_ROW_TILES):
                pm = metas[:, rt, 0:1]
                u = upool.tile([128, B], FP, tag="u")
                pool = smp.tile([128, N_CHUNKS * 8], FP, tag="pool")

                for c in range(N_CHUNKS):
                    ps = psp.tile([128, CHUNK], FP, tag="ps")
                    for k in range(NK):
                        nc.tensor.matmul(
                            ps[:],
                            eloc_t[k][:, rt * 128:(rt + 1) * 128],
                            et_t[k][:, c * CHUNK:(c + 1) * CHUNK],
                            start=(k == 0),
                            stop=(k == NK - 1),
                        )
                    uc = u[:, c * CHUNK:(c + 1) * CHUNK]
                    nc.scalar.copy(uc, ps[:])
                    pen = penp.tile([128, CHUNK], FP, tag="pen")
                    nc.vector.tensor_scalar(
                        pen[:], uc, pm, -GESHIFT,
                        op0=mybir.AluOpType.is_ge, op1=mybir.AluOpType.mult,
                    )
                    nc.gpsimd.tensor_tensor(uc, uc, pen[:], op=mybir.AluOpType.add)
                    nc.vector.max(pool[:, c * 8:(c + 1) * 8], uc)

                # merge 16 segment top-8s -> top-16
                poolmr = smp.tile([128, N_CHUNKS * 8], FP, tag="poolmr")
                nc.vector.max(t16a[:, rt, 0:8], pool[:])
                nc.vector.match_replace(poolmr[:], t16a[:, rt, 0:8], pool[:],
                                        -32768.0)
                nc.vector.max(t16a[:, rt, 8:16], poolmr[:])

            # deferred epilogue: ACT loads Exp/Ln tables exactly once
            for rt in range(N_ROW_TILES):
                psim = metas[:, rt, 1:2]
                val = metas[:, rt, 2:3]
                top16 = t16a[:, rt, :]
                m = t16a[:, rt, 0:1]
                hs = smp.tile([128, 1], FP, tag="hs")
                nc.vector.tensor_scalar(
                    hs[:], m, -500.0, None, op0=mybir.AluOpType.is_gt)
                corr = smp.tile([128, 1], FP, tag="corr")
                nc.vector.tensor_scalar(
                    corr[:], hs[:], -CORR, CORR,
                    op0=mybir.AluOpType.mult, op1=mybir.AluOpType.add)
                bneg = smp.tile([128, 1], FP, tag="bneg")
                nc.vector.tensor_scalar(
                    bneg[:], m, -1.0 / TEMP, None, op0=mybir.AluOpType.mult)
                e16 = smp.tile([128, 16], FP, tag="e16")
                sume = smp.tile([128, 1], FP, tag="sume")
                nc.scalar.activation(
                    e16[:], top16, mybir.ActivationFunctionType.Exp,
                    bias=bneg[:], scale=1.0 / TEMP, accum_out=sume[:])
                # drop the bogus self term for has_semi=False rows:
                # sume_eff = sume - (1 - hs)  (its exp term is exactly 1.0)
                hsm1 = smp.tile([128, 1], FP, tag="hsm1")
                nc.vector.tensor_scalar(
                    hsm1[:], hs[:], 1.0, None, op0=mybir.AluOpType.subtract)
                sume2 = smp.tile([128, 1], FP, tag="sume2")
                nc.vector.tensor_tensor(
                    sume2[:], sume[:], hsm1[:], op=mybir.AluOpType.add)
                nc.vector.tensor_scalar(
                    sume2[:], sume2[:], 1e-30, None, op0=mybir.AluOpType.max)
                lnz = smp.tile([128, 1], FP, tag="lnz")
                nc.scalar.activation(
                    lnz[:], sume2[:], mybir.ActivationFunctionType.Ln)
                # loss = (m/T + lnz + corr - psim) * val
                a1 = smp.tile([128, 1], FP, tag="a1")
                nc.vector.tensor_scalar(
                    a1[:], m, 1.0 / TEMP, None, op0=mybir.AluOpType.mult)
                nc.vector.tensor_tensor(a1[:], a1[:], lnz[:], op=mybir.AluOpType.add)
                nc.vector.tensor_tensor(a1[:], a1[:], corr[:], op=mybir.AluOpType.add)
                nc.vector.tensor_tensor(a1[:], a1[:], psim, op=mybir.AluOpType.subtract)
                nc.vector.tensor_tensor(
                    loss_t[:, rt:rt + 1], a1[:], val, op=mybir.AluOpType.mult)

            nc.sync.dma_start(out_d[:], loss_t[:])

    nc.compile()
    return nc


def _host_rowmeta(emb: np.ndarray, labels: np.ndarray):
    """pos_min / pos_sim / valid per row from label groups (tiny)."""
    Bn = emb.shape[0]
    pos_min = np.full(Bn, 1e30, np.float32)
    pos_sum = np.zeros(Bn, np.float32)
    cnt = np.zeros(Bn, np.int64)
    order = np.argsort(labels, kind="stable")
    sl = labels[order]
    starts = np.flatnonzero(np.r_[True, sl[1:] != sl[:-1]])
    ends = np.r_[starts[1:], Bn]
    for s, e in zip(starts, ends):
        idx = order[s:e]
        n = e - s
        if n < 2:
            continue
        G = emb[idx] @ emb[idx].T          # [n, n] fp32
        np.fill_diagonal(G, np.nan)
        pos_min[idx] = np.nanmin(G, axis=1)
        pos_sum[idx] = np.nansum(G, axis=1)
        cnt[idx] = n - 1
    pos_sim = pos_sum / np.maximum(cnt, 1) / TEMP
    valid = ((cnt > 0) & ((Bn - 1 - cnt) > 0)).astype(np.float32)
    meta = np.zeros((Bn, 4), np.float32)
    meta[:, 0] = pos_min
    meta[:, 1] = pos_sim
    meta[:, 2] = valid
    return meta, valid.sum()


_profile = [None]


def kernel(embeddings: np.ndarray, labels: np.ndarray) -> np.ndarray:
    emb = np.asarray(embeddings, np.float32)
    lab = np.asarray(labels)
    meta, n_valid = _host_rowmeta(emb, lab)

    et = np.ascontiguousarray(emb.T).astype(mybir.dt.np(BF))          # [D, B] bf16
    in_maps = []
    for c in range(N_CORES):
        r0 = c * ROWS_PER_CORE
        in_maps.append({
            "et": et,
            "eloc": np.ascontiguousarray(emb[r0:r0 + ROWS_PER_CORE].T)
                      .astype(mybir.dt.np(BF)),
            "rowmeta": meta[r0:r0 + ROWS_PER_CORE],
        })

    nc = _build_program()
    trace = _profile[0] is not None
    res = run_bass_kernel_spmd(nc, in_maps, list(range(N_CORES)), trace=trace)
    if trace:
        _profile[0] = res
    total = np.float64(0.0)
    for c in range(N_CORES):
        total += np.asarray(res.results[c]["out"], np.float64).sum()
    return np.float32(total / max(n_valid, 1.0))



# revision 4
# speedup vs baseline: 1.8371x; 1.8371x over previous
"""HardNegativeMiningLoss on 8 TRN2 NeuronCores.

Data-parallel over anchor rows: core c owns rows [1024c, 1024(c+1)).
Each core holds full E^T (fp16) in SBUF and computes its [1024, 8192]
sim block with TensorE into 2048-wide PSUM tiles (fp32, half of PSUM,
double buffered).  ScalarE evacuates each 2048-block to fp16 SBUF in
one wide ACT copy.  VectorE applies the semi-hard shift
u - 8*[u >= pos_min] (tensor_scalar is_ge/mult at 4x fp16 rate +
in-place tensor_tensor add at 2x), folds the block 2048->512 with two
packed tensor_tensor max ops, and takes the per-block top-8 with MAX8.
Per row-tile the 4 blocks' top-8s merge to top-16 via
max8/match_replace/max8.  The masked logsumexp epilogue is deferred and
batched so ACT loads the Exp and Ln tables exactly once.  Label-derived
row metadata (pos_min / pos_sim / valid) is precomputed on host
(~0.05% of FLOPs).  Host sums the per-core [128, 8] partials.
"""

import numpy as np

import concourse.bacc as bacc
import concourse.bass as bass
import concourse.mybir as mybir
import concourse.tile as tile
from concourse.bass_utils import run_bass_kernel_spmd

B = 8192
D = 512
N_CORES = 8
ROWS_PER_CORE = B // N_CORES          # 1024
N_ROW_TILES = ROWS_PER_CORE // 128    # 8
BLK = 2048
N_BLKS = B // BLK                     # 4
SUB = 512                             # psum quarter (one bank)
N_SUBS = BLK // SUB                   # 4
TEMP = 0.07
SHIFT = 8.0
CORR = SHIFT / TEMP
FP = mybir.dt.float32
F16 = mybir.dt.float16
NK = D // 128                         # 4


def _build_program():
    nc = bacc.Bacc(None, target_bir_lowering=False)

    et_d = nc.dram_tensor("et", [D, B], F16, kind="ExternalInput")
    eloc_d = nc.dram_tensor("eloc", [D, ROWS_PER_CORE], F16, kind="ExternalInput")
    meta_d = nc.dram_tensor("rowmeta", [ROWS_PER_CORE, 4], FP, kind="ExternalInput")
    out_d = nc.dram_tensor("out", [128, N_ROW_TILES], FP, kind="ExternalOutput")

    et_v = et_d[:].rearrange("(k p) n -> k p n", p=128)       # [4,128,B]
    eloc_v = eloc_d[:].rearrange("(k p) n -> k p n", p=128)   # [4,128,1024]
    meta_v = meta_d[:].rearrange("(t p) m -> p t m", p=128)   # [128,8,4]

    with tile.TileContext(nc) as tc:
        with (
            tc.tile_pool(name="wts", bufs=1) as wts,
            tc.tile_pool(name="psum", bufs=2, space="PSUM") as psp,
            tc.tile_pool(name="ub", bufs=3) as ubp,
            tc.tile_pool(name="pen", bufs=3) as penp,
            tc.tile_pool(name="f1", bufs=2) as f1p,
            tc.tile_pool(name="f2", bufs=2) as f2p,
            tc.tile_pool(name="small", bufs=2) as smp,
            tc.tile_pool(name="acc", bufs=1) as accp,
        ):
            # resident inputs
            metas = accp.tile([128, N_ROW_TILES, 4], FP, tag="metas")
            nc.sync.dma_start(metas[:], meta_v)
            eloc_t = []
            for k in range(NK):
                t = wts.tile([128, ROWS_PER_CORE], F16, tag=f"el{k}")
                nc.sync.dma_start(t[:], eloc_v[k])
                eloc_t.append(t)
            et_t = [wts.tile([128, B], F16, tag=f"et{k}", name=f"et{k}")
                    for k in range(NK)]
            # column-piece DMAs so the first block's matmuls start early
            for p in range(N_BLKS):
                for k in range(NK):
                    nc.sync.dma_start(
                        et_t[k][:, p * BLK:(p + 1) * BLK],
                        et_v[k][:, p * BLK:(p + 1) * BLK],
                    )

            pool = accp.tile([128, N_ROW_TILES, N_BLKS * 8], F16, tag="pool")
            t16 = accp.tile([128, N_ROW_TILES, 16], F16, tag="t16")

            for rt in range(N_ROW_TILES):
                pm = metas[:, rt, 0:1]
                for blk in range(N_BLKS):
                    ps = psp.tile([128, BLK], FP, tag="ps")
                    for k in range(NK):
                        lhsT = eloc_t[k][:, rt * 128:(rt + 1) * 128]
                        for c in range(N_SUBS):
                            col0 = blk * BLK + c * SUB
                            nc.tensor.matmul(
                                ps[:, c * SUB:(c + 1) * SUB],
                                lhsT,
                                et_t[k][:, col0:col0 + SUB],
                                start=(k == 0),
                                stop=(k == NK - 1),
                            )
                    ub = ubp.tile([128, BLK], F16, tag="ub")
                    nc.scalar.copy(ub[:], ps[:])
                    pen = penp.tile([128, BLK], F16, tag="pen")
                    nc.vector.tensor_scalar(
                        pen[:], ub[:], pm, -SHIFT,
                        op0=mybir.AluOpType.is_ge, op1=mybir.AluOpType.mult,
                    )
                    nc.vector.tensor_tensor(
                        ub[:], ub[:], pen[:], op=mybir.AluOpType.add)
                    f1 = f1p.tile([128, BLK // 2], F16, tag="f1")
                    nc.vector.tensor_tensor(
                        f1[:], ub[:, 0:BLK // 2], ub[:, BLK // 2:BLK],
                        op=mybir.AluOpType.max)
                    f2 = f2p.tile([128, BLK // 4], F16, tag="f2")
                    nc.vector.tensor_tensor(
                        f2[:], f1[:, 0:BLK // 4], f1[:, BLK // 4:BLK // 2],
                        op=mybir.AluOpType.max)
                    nc.vector.max(pool[:, rt, blk * 8:(blk + 1) * 8], f2[:])

                # merge 4 block top-8s -> top-16
                mr = smp.tile([128, N_BLKS * 8], F16, tag="mr")
                nc.vector.max(t16[:, rt, 0:8], pool[:, rt, :])
                nc.vector.match_replace(mr[:], t16[:, rt, 0:8], pool[:, rt, :],
                                        -32768.0)
                nc.vector.max(t16[:, rt, 8:16], mr[:])

            # deferred batched epilogue: ACT loads Exp then Ln exactly once
            m8 = accp.tile([128, N_ROW_TILES], FP, tag="m8")
            nc.vector.tensor_scalar(
                m8[:], t16[:, :, 0], 1.0, None, op0=mybir.AluOpType.mult)
            x32 = accp.tile([128, N_ROW_TILES, 16], FP, tag="x32")
            for rt in range(N_ROW_TILES):
                nc.vector.tensor_scalar(
                    x32[:, rt, :], t16[:, rt, :], m8[:, rt:rt + 1], 1.0 / TEMP,
                    op0=mybir.AluOpType.subtract, op1=mybir.AluOpType.mult,
                )
            e32 = accp.tile([128, N_ROW_TILES, 16], FP, tag="e32")
            nc.scalar.activation(e32[:], x32[:], mybir.ActivationFunctionType.Exp)
            sume8 = accp.tile([128, N_ROW_TILES], FP, tag="sume8")
            nc.vector.tensor_reduce(
                sume8[:], e32[:], axis=mybir.AxisListType.X,
                op=mybir.AluOpType.add)
            hs8 = accp.tile([128, N_ROW_TILES], FP, tag="hs8")
            nc.vector.tensor_scalar(
                hs8[:], m8[:], -4.0, None, op0=mybir.AluOpType.is_gt)
            # sume2 = sume8 + hs8 - 1  (drop the bogus self term when no semi)
            sume2 = accp.tile([128, N_ROW_TILES], FP, tag="sume2")
            nc.vector.scalar_tensor_tensor(
                sume2[:], sume8[:], -1.0, hs8[:],
                op0=mybir.AluOpType.add, op1=mybir.AluOpType.add)
            nc.vector.tensor_scalar(
                sume2[:], sume2[:], 1e-30, None, op0=mybir.AluOpType.max)
            lnz8 = accp.tile([128, N_ROW_TILES], FP, tag="lnz8")
            nc.scalar.activation(lnz8[:], sume2[:],
                                 mybir.ActivationFunctionType.Ln)
            corr8 = accp.tile([128, N_ROW_TILES], FP, tag="corr8")
            nc.vector.tensor_scalar(
                corr8[:], hs8[:], -CORR, CORR,
                op0=mybir.AluOpType.mult, op1=mybir.AluOpType.add)
            # loss = (m/T + lnz + corr - psim) * valid
            a8 = accp.tile([128, N_ROW_TILES], FP, tag="a8")
            nc.vector.tensor_scalar(
                a8[:], m8[:], 1.0 / TEMP, None, op0=mybir.AluOpType.mult)
            nc.vector.tensor_tensor(a8[:], a8[:], lnz8[:],
                                    op=mybir.AluOpType.add)
            nc.vector.tensor_tensor(a8[:], a8[:], corr8[:],
                                    op=mybir.AluOpType.add)
            nc.vector.tensor_tensor(a8[:], a8[:], metas[:, :, 1],
                                    op=mybir.AluOpType.subtract)
            loss8 = accp.tile([128, N_ROW_TILES], FP, tag="loss8")
            nc.vector.tensor_tensor(loss8[:], a8[:], metas[:, :, 2],
                                    op=mybir.AluOpType.mult)

            nc.sync.dma_start(out_d[:], loss8[:])

    nc.compile()
    return nc


def _host_rowmeta(emb: np.ndarray, labels: np.ndarray):
    """pos_min / pos_sim / valid per row from label groups (tiny)."""
    Bn = emb.shape[0]
    pos_min = np.full(Bn, 1e30, np.float32)
    pos_sum = np.zeros(Bn, np.float32)
    cnt = np.zeros(Bn, np.int64)
    order = np.argsort(labels, kind="stable")
    sl = labels[order]
    starts = np.flatnonzero(np.r_[True, sl[1:] != sl[:-1]])
    ends = np.r_[starts[1:], Bn]
    for s, e in zip(starts, ends):
        idx = order[s:e]
        n = e - s
        if n < 2:
            continue
        G = emb[idx] @ emb[idx].T          # [n, n] fp32
        np.fill_diagonal(G, np.nan)
        pos_min[idx] = np.nanmin(G, axis=1)
        pos_sum[idx] = np.nansum(G, axis=1)
        cnt[idx] = n - 1
    pos_sim = pos_sum / np.maximum(cnt, 1) / TEMP
    valid = ((cnt > 0) & ((Bn - 1 - cnt) > 0)).astype(np.float32)
    meta = np.zeros((Bn, 4), np.float32)
    meta[:, 0] = pos_min
    meta[:, 1] = pos_sim
    meta[:, 2] = valid
    return meta, valid.sum()


_profile = [None]


def kernel(embeddings: np.ndarray, labels: np.ndarray) -> np.ndarray:
    emb = np.asarray(embeddings, np.float32)
    lab = np.asarray(labels)
    meta, n_valid = _host_rowmeta(emb, lab)

    et = np.ascontiguousarray(emb.T).astype(np.float16)       # [D, B] fp16
    in_maps = []
    for c in range(N_CORES):
        r0 = c * ROWS_PER_CORE
        in_maps.append({
            "et": et,
            "eloc": np.ascontiguousarray(emb[r0:r0 + ROWS_PER_CORE].T)
                      .astype(np.float16),
            "rowmeta": meta[r0:r0 + ROWS_PER_CORE],
        })

    nc = _build_program()
    trace = _profile[0] is not None
    res = run_bass_kernel_spmd(nc, in_maps, list(range(N_CORES)), trace=trace)
    if trace:
        _profile[0] = res
    total = np.float64(0.0)
    for c in range(N_CORES):
        total += np.asarray(res.results[c]["out"], np.float64).sum()
    return np.float32(total / max(n_valid, 1.0))
